# revision 30
# baseline (speedup 1.0000x reference)
"""Multi-head self-attention Trainium2 kernel (Bass/Tile).

Problem: x:(8,256,32,32), 8 heads, head_dim=32, N=H*W=1024.
Sharding: data-parallel over batch B=8 -> one batch element per NeuronCore.

Per-core math (b fixed, X = x[b] as (C=256, N=1024)):
  q = Wq@X + bq ; k = Wk@X + bk ; v = Wv@X + bv      (per-pixel linear)
  S[n,m] = sum_d q[d,n]k[d,m] / sqrt(32)  (per head)
  P = softmax_m(S) ; O[d,n] = sum_m P[n,m] v[d,m] ; out = Wo@O + bo + X

Bias algebra (all exact, folded on host):
  - bk contributes q^T bk, constant along the softmax axis -> drops.
  - bq contributes (bq^T k_raw)[m]: folded in as an extra row of an
    augmented K-hat projection (row u_h = Wk_h^T bq_h / sqrt32), matched by a
    ones-row in Q-hat -> scores come out of the PE fully biased+scaled, so
    exp needs no per-partition bias and one ACT op can span 2 packed heads.
  - bv contributes bv * sum_m P = bv -> folded into residual via
    xpb = x[b] + (Wo@bv + bo).
  - 1/sqrt(32) folded into Wq-hat and u rows.

Layouts (per core):
  Qh/Kh: 8 head-slabs of 64 rows (head h at rows 64h..64h+33; Q row 32 = ones,
         K row 32 = bqk row), stored as 4 SBUF tiles [128,1024].
  S^T computed per (head-pair, m-chunk, n-half): psum [128 m, 2x512 n] with
  heads A (array rows 0-32) and B (rows 64-96) packed concurrently.
  exp on ScalarE (no bias) -> E^T [128,1024] SBUF.
  V-hat: [128, 8*33] per m-chunk: per head 32 V^T cols + a ones col, so the
  AV matmul (M=33) yields O-tilde rows 0..31 and the softmax denominator in
  row 32 for free. Normalization applied to O (1024*32 per head), not E.
"""

import math
import os

import numpy as np

import concourse.bass as bass
import concourse.mybir as mybir
import concourse.tile as tile
from concourse import bacc
from concourse.bass_utils import run_bass_kernel_spmd

F32 = mybir.dt.float32
F32R = mybir.dt.float32r
BF16 = mybir.dt.bfloat16
EXP = mybir.ActivationFunctionType.Exp

NH = 8          # heads
HD = 32         # head dim
C = 256         # channels
N = 1024        # H*W
NCORES = 8

_NC = None          # cached compiled Bass module
LAST_RESULTS = None  # BassKernelResults of most recent run (for test.py)


def _r(ap):
    return ap.bitcast(F32R)


def _emit(tc, io):
    nc = tc.nc
    import contextlib

    ctx = contextlib.ExitStack()
    with ctx:
        pers = ctx.enter_context(tc.tile_pool(name="pers", bufs=1))
        etp = ctx.enter_context(tc.tile_pool(name="etp", bufs=6))
        psp = ctx.enter_context(tc.tile_pool(name="psp", bufs=2, space="PSUM"))

        def ptile(name, shape, dtype=F32):
            return pers.tile(shape, dtype, tag=name, name=name)

        # ---------------- load inputs ----------------
        X = [ptile(f"X{i}", [128, N], F32R) for i in range(2)]
        XPB = [ptile(f"XPB{i}", [128, N]) for i in range(2)]
        WQT = [ptile(f"WQT{i}", [128, 512], F32R) for i in range(2)]
        WKT = [ptile(f"WKT{i}", [128, 512], F32R) for i in range(2)]
        WVT = [ptile(f"WVT{i}", [128, C], F32R) for i in range(2)]
        WOT = [ptile(f"WOT{i}", [128, C], F32R) for i in range(2)]
        for half in range(2):  # first halves land first -> earlier first matmul
            for i in range(2):
                sl = slice(i * 128, (i + 1) * 128)
                hw_ = slice(half * 256, (half + 1) * 256)
                hn = slice(half * 512, (half + 1) * 512)
                nc.sync.dma_start(X[i][:, hn], io["xb"][sl, hn])
                nc.sync.dma_start(WQT[i][:, hw_], io["wqt"][sl, hw_])
                nc.sync.dma_start(WKT[i][:, hw_], io["wkt"][sl, hw_])
                if half == 0:
                    nc.sync.dma_start(WVT[i][:], io["wvt"][sl, :])

        # ---------------- Q-hat / K-hat projections ----------------
        # padded channel space: head h -> rows 64h..64h+33 (4 tiles of 128)
        Qh = [ptile(f"Qh{t}", [128, N], F32R) for t in range(4)]
        Kh = [ptile(f"Kh{t}", [128, N], F32R) for t in range(4)]
        for t in range(4):
            for dst, w in ((Qh, WQT), (Kh, WKT)):
                pp = psp.tile([128, N], F32, tag="big", bufs=3, name=f"pp_{t}")
                for jn in range(2):
                    for kc in range(2):
                        nc.tensor.matmul(
                            pp[:, jn * 512 : (jn + 1) * 512],
                            (w[kc][:, t * 128 : (t + 1) * 128]),
                            (X[kc][:, jn * 512 : (jn + 1) * 512]),
                            start=(kc == 0),
                            stop=(kc == 1),
                        )
                nc.vector.tensor_copy(dst[t][:], pp[:])
            # ones rows for Q-hat (row 32 of each 64-row slab)
            nc.gpsimd.memset(Qh[t][32:33, :].bitcast(F32), 1.0)
            nc.gpsimd.memset(Qh[t][96:97, :].bitcast(F32), 1.0)

        # ---------------- V^T (ones-augmented) ----------------
        # VH[mc][:, 33h:33h+32] = V^T rows mc*128.., head h; col 33h+32 = 1.0
        VH = [ptile(f"VH{mc}", [128, NH * 33], BF16) for mc in range(NH)]
        for mc in range(8):
            pv = psp.tile([128, C], F32, tag="big", bufs=3, name=f"pv_{mc}")
            for kc in range(2):
                nc.tensor.matmul(
                    pv[:],
                    (X[kc][:, mc * 128 : (mc + 1) * 128]),
                    (WVT[kc][:]),
                    start=(kc == 0),
                    stop=(kc == 1),
                )
            nc.vector.memset(VH[mc][:], 1.0)  # ones cols survive the copy below
            vh3 = VH[mc].rearrange("p (h c) -> p h c", c=33)
            nc.vector.tensor_copy(
                vh3[:, :, 0:32], pv.rearrange("p (h d) -> p h d", d=32)
            )

        # ---------------- attention ----------------
        O1u = [ptile(f"O1u{t}", [128, N]) for t in range(2)]
        ESUM = ptile("ESUM", [36, N])  # heads 0-3 rows 0-3; 4-7 rows 32-35
        nc.vector.memset(ESUM[:], 1.0)  # unused rows stay 1.0 -> 1/x finite
        RECIP = ptile("RECIP", [36, N], F32R)
        OH = ptile("OH", [36, C], F32R)
        nc.sync.dma_start(OH[:], io["oh"][:, :])
        for i in range(2):
            sl = slice(i * 128, (i + 1) * 128)
            nc.sync.dma_start(WOT[i][:], io["wot"][sl, :])
            nc.sync.dma_start(XPB[i][:], io["xpb"][sl, :])
        O1 = [ptile(f"O1{t}", [128, N], F32R) for t in range(2)]
        for p in range(4):  # head pairs (2p, 2p+1) in Qh/Kh tile p
            # one [97,512] psum per jn: head A rows 0-32 (array cols 0-32),
            # head B rows 64-96 (array cols 64-96) -> the two AV matmuls run
            # on disjoint column groups concurrently.
            psO = [
                psp.tile([97, 512], F32, tag="psO", bufs=2, name=f"psO_{p}_{jn}")
                for jn in range(2)
            ]
            for mc in range(8):
                for jn in range(2):
                    ps = psp.tile([128, N], F32, tag="big", bufs=3, name=f"ps_{p}_{mc}_{jn}")
                    for hh in range(2):  # array rows 0-32 / 64-96: concurrent
                        base = 64 * hh
                        nc.tensor.matmul(
                            ps[:, hh * 512 : (hh + 1) * 512],
                            (Kh[p][base : base + 33, mc * 128 : (mc + 1) * 128]),
                            (Qh[p][base : base + 33, jn * 512 : (jn + 1) * 512]),
                            start=True,
                            stop=True,
                        )
                    et = etp.tile([128, N], BF16, tag="et", name=f"et_{p}_{mc}_{jn}")
                    nc.scalar.activation(et[:], ps[:], EXP)
                    for hh in range(2):
                        h = 2 * p + hh
                        nc.tensor.matmul(
                            psO[jn][64 * hh : 64 * hh + 33, :],
                            (VH[mc][:, 33 * h : 33 * h + 33]),
                            (et[:, hh * 512 : (hh + 1) * 512]),
                            start=(mc == 0),
                            stop=(mc == 7),
                            tile_position=(0, 64 * hh),
                            skip_group_check=True,
                        )
            for jn in range(2):
                js = slice(jn * 512, (jn + 1) * 512)
                ost = etp.tile([97, 512], F32, tag="ost", bufs=4, name=f"ost_{p}_{jn}")
                nc.vector.tensor_copy(ost[0:33, :], psO[jn][0:33, :])
                nc.vector.tensor_copy(ost[64:97, :], psO[jn][64:97, :])
                for hh in range(2):
                    h = 2 * p + hh
                    t, r = h // 4, 32 * (h % 4)
                    nc.sync.dma_start(
                        O1u[t][r : r + 32, js], ost[64 * hh : 64 * hh + 32, :]
                    )
                    er = 32 * (h // 4) + h % 4
                    nc.sync.dma_start(
                        ESUM[er : er + 1, js], ost[64 * hh + 32 : 64 * hh + 33, :]
                    )



        # ---------------- normalize ----------------
        with nc.allow_low_precision("f32r recip of O(100) sums"):
            nc.vector.reciprocal(RECIP[:], ESUM[:])  # one FD-bound op covers both halves
        for t in range(2):
            rs = slice(32 * t, 32 * t + 4)
            pr = psp.tile([128, N], F32, tag="big", bufs=3, name=f"pr_{t}")
            for jn in range(2):
                js = slice(jn * 512, (jn + 1) * 512)
                nc.tensor.matmul(
                    pr[:, js],
                    (OH[rs, t * 128 : (t + 1) * 128]),
                    (RECIP[rs, js]),
                    start=True,
                    stop=True,
                )
            nc.vector.tensor_mul(O1[t][:], O1u[t][:], pr[:])

        # ---------------- output projection + residual ----------------
        OUTF = [ptile(f"OUTF{t}", [128, N]) for t in range(2)]
        for mo in range(2):
            for jn in range(2):
                js = slice(jn * 512, (jn + 1) * 512)
                po = psp.tile([128, 512], F32, tag="big", bufs=3, name=f"po_{mo}_{jn}")
                for kc in range(2):
                    nc.tensor.matmul(
                        po[:],
                        (WOT[kc][:, mo * 128 : (mo + 1) * 128]),
                        (O1[kc][:, js]),
                        start=(kc == 0),
                        stop=(kc == 1),
                    )
                nc.vector.tensor_add(OUTF[mo][:, js], po[:], XPB[mo][:, js])
            nc.sync.dma_start(io["out"][mo * 128 : (mo + 1) * 128, :], OUTF[mo][:])


def build_nc():
    nc = bacc.Bacc("TRN2", target_bir_lowering=False, debug=False)
    io = {}
    for name, shape, dt_ in [
        ("xb", (C, N), F32R),
        ("xpb", (C, N), F32),
        ("wqt", (C, 512), F32R),
        ("wkt", (C, 512), F32R),
        ("wvt", (C, C), F32R),
        ("wot", (C, C), F32R),
        ("oh", (36, C), F32R),
    ]:
        io[name] = nc.dram_tensor(name, shape, dt_, kind="ExternalInput").ap()
    io["out"] = nc.dram_tensor("out", (C, N), F32, kind="ExternalOutput").ap()
    with tile.TileContext(nc) as tc:
        _emit(tc, io)
    nc.finalize()  # Bacc passes: wait-splitting (1-wait limit), reg alloc
    return nc


def host_prep(x, Wq, bq, Wk, bk, Wv, bv, Wo, bo):
    """Build per-core input maps (numpy only)."""
    x = np.ascontiguousarray(np.asarray(x, np.float32))
    Wq, bq = np.asarray(Wq, np.float32), np.asarray(bq, np.float32)
    Wk = np.asarray(Wk, np.float32)
    Wv, bv = np.asarray(Wv, np.float32), np.asarray(bv, np.float32)
    Wo, bo = np.asarray(Wo, np.float32), np.asarray(bo, np.float32)
    s = 1.0 / math.sqrt(HD)

    wqt = np.zeros((C, 512), np.float32)
    wkt = np.zeros((C, 512), np.float32)
    for h in range(NH):
        hs = slice(HD * h, HD * (h + 1))
        wqt[:, 64 * h : 64 * h + 32] = Wq[hs, :].T * s
        wkt[:, 64 * h : 64 * h + 32] = Wk[hs, :].T
        wkt[:, 64 * h + 32] = (Wk[hs, :].T @ bq[hs]) * s
    wvt = np.ascontiguousarray(Wv.T)
    wot = np.ascontiguousarray(Wo.T)
    bo2 = Wo @ bv + bo
    # oh[32t + j//32, t*128 + j] = 1: selects head rows for the
    # recip-broadcast matmul (RECIP row layout: heads 0-3 at rows 0-3,
    # heads 4-7 at rows 32-35 for partition-aligned matmul slices).
    oh = np.zeros((36, C), np.float32)
    for t in range(2):
        for j in range(128):
            oh[32 * t + j // 32, t * 128 + j] = 1.0

    B = x.shape[0]
    in_maps = []
    for b in range(B):
        xb = np.ascontiguousarray(x[b].reshape(C, N))
        in_maps.append(
            {
                "xb": xb,
                "xpb": np.ascontiguousarray(xb + bo2[:, None]),
                "wqt": wqt,
                "wkt": wkt,
                "wvt": wvt,
                "wot": wot,
                "oh": oh,
            }
        )
    return in_maps


def kernel(x, Wq, bq, Wk, bk, Wv, bv, Wo, bo):
    global _NC, LAST_RESULTS
    if _NC is None:
        _NC = build_nc()
    in_maps = host_prep(x, Wq, bq, Wk, bk, Wv, bv, Wo, bo)
    res = run_bass_kernel_spmd(_NC, in_maps, core_ids=list(range(NCORES)))
    LAST_RESULTS = res
    out = np.stack([r["out"] for r in res.results], axis=0)
    return out.reshape(NCORES, C, 32, 32).astype(np.float32)


if __name__ == "__main__":
    # smoke: random inputs through the kernel
    rng = np.random.default_rng(0)
    ins = {
        "x": rng.standard_normal((8, C, 32, 32), dtype=np.float32),
        "Wq": rng.standard_normal((C, C), dtype=np.float32) / 16,
        "bq": rng.standard_normal(C).astype(np.float32) * 0.01,
        "Wk": rng.standard_normal((C, C), dtype=np.float32) / 16,
        "bk": rng.standard_normal(C).astype(np.float32) * 0.01,
        "Wv": rng.standard_normal((C, C), dtype=np.float32) / 16,
        "bv": rng.standard_normal(C).astype(np.float32) * 0.01,
        "Wo": rng.standard_normal((C, C), dtype=np.float32) / 16,
        "bo": rng.standard_normal(C).astype(np.float32) * 0.01,
    }
    out = kernel(**ins)
    print("out", out.shape, out.dtype, float(np.abs(out).mean()))


# revision 31
# speedup vs baseline: 1.0008x; 1.0008x over previous
"""Multi-head self-attention Trainium2 kernel (Bass/Tile).

Problem: x:(8,256,32,32), 8 heads, head_dim=32, N=H*W=1024.
Sharding: data-parallel over batch B=8 -> one batch element per NeuronCore.

Per-core math (b fixed, X = x[b] as (C=256, N=1024)):
  q = Wq@X + bq ; k = Wk@X + bk ; v = Wv@X + bv      (per-pixel linear)
  S[n,m] = sum_d q[d,n]k[d,m] / sqrt(32)  (per head)
  P = softmax_m(S) ; O[d,n] = sum_m P[n,m] v[d,m] ; out = Wo@O + bo + X

Bias algebra (all exact, folded on host):
  - bk contributes q^T bk, constant along the softmax axis -> drops.
  - bq contributes (bq^T k_raw)[m]: folded in as an extra row of an
    augmented K-hat projection (row u_h = Wk_h^T bq_h / sqrt32), matched by a
    ones-row in Q-hat -> scores come out of the PE fully biased+scaled, so
    exp needs no per-partition bias and one ACT op can span 2 packed heads.
  - bv contributes bv * sum_m P = bv -> folded into residual via
    xpb = x[b] + (Wo@bv + bo).
  - 1/sqrt(32) folded into Wq-hat and u rows.

Layouts (per core):
  Qh/Kh: 8 head-slabs of 64 rows (head h at rows 64h..64h+33; Q row 32 = ones,
         K row 32 = bqk row), stored as 4 SBUF tiles [128,1024].
  S^T computed per (head-pair, m-chunk, n-half): psum [128 m, 2x512 n] with
  heads A (array rows 0-32) and B (rows 64-96) packed concurrently.
  exp on ScalarE (no bias) -> E^T [128,1024] SBUF.
  V-hat: [128, 8*33] per m-chunk: per head 32 V^T cols + a ones col, so the
  AV matmul (M=33) yields O-tilde rows 0..31 and the softmax denominator in
  row 32 for free. Normalization applied to O (1024*32 per head), not E.
"""

import math
import os

import numpy as np

import concourse.bass as bass
import concourse.mybir as mybir
import concourse.tile as tile
from concourse import bacc
from concourse.bass_utils import run_bass_kernel_spmd

F32 = mybir.dt.float32
F32R = mybir.dt.float32r
BF16 = mybir.dt.bfloat16
EXP = mybir.ActivationFunctionType.Exp

NH = 8          # heads
HD = 32         # head dim
C = 256         # channels
N = 1024        # H*W
NCORES = 8

_NC = None          # cached compiled Bass module
LAST_RESULTS = None  # BassKernelResults of most recent run (for test.py)


def _r(ap):
    return ap.bitcast(F32R)


def _emit(tc, io):
    nc = tc.nc
    import contextlib

    ctx = contextlib.ExitStack()
    with ctx:
        pers = ctx.enter_context(tc.tile_pool(name="pers", bufs=1))
        etp = ctx.enter_context(tc.tile_pool(name="etp", bufs=6))
        psp = ctx.enter_context(tc.tile_pool(name="psp", bufs=2, space="PSUM"))

        def ptile(name, shape, dtype=F32):
            return pers.tile(shape, dtype, tag=name, name=name)

        # ---------------- load inputs ----------------
        X = [ptile(f"X{i}", [128, N], F32R) for i in range(2)]
        XPB = [ptile(f"XPB{i}", [128, N]) for i in range(2)]
        WQT = [ptile(f"WQT{i}", [128, 512], F32R) for i in range(2)]
        WKT = [ptile(f"WKT{i}", [128, 512], F32R) for i in range(2)]
        WVT = [ptile(f"WVT{i}", [128, C], F32R) for i in range(2)]
        WOT = [ptile(f"WOT{i}", [128, C], F32R) for i in range(2)]
        for i in range(2):
            sl = slice(i * 128, (i + 1) * 128)
            nc.sync.dma_start(X[i][:], io["xb"][sl, :])
            nc.sync.dma_start(WQT[i][:], io["wqt"][sl, :])
            nc.sync.dma_start(WKT[i][:], io["wkt"][sl, :])
            nc.sync.dma_start(WVT[i][:], io["wvt"][sl, :])

        # ---------------- Q-hat / K-hat projections ----------------
        # padded channel space: head h -> rows 64h..64h+33 (4 tiles of 128)
        Qh = [ptile(f"Qh{t}", [128, N], F32R) for t in range(4)]
        Kh = [ptile(f"Kh{t}", [128, N], F32R) for t in range(4)]
        for t in range(4):
            for dst, w in ((Qh, WQT), (Kh, WKT)):
                pp = psp.tile([128, N], F32, tag="big", bufs=3, name=f"pp_{t}")
                for jn in range(2):
                    for kc in range(2):
                        nc.tensor.matmul(
                            pp[:, jn * 512 : (jn + 1) * 512],
                            (w[kc][:, t * 128 : (t + 1) * 128]),
                            (X[kc][:, jn * 512 : (jn + 1) * 512]),
                            start=(kc == 0),
                            stop=(kc == 1),
                        )
                nc.vector.tensor_copy(dst[t][:], pp[:])
            # ones rows for Q-hat (row 32 of each 64-row slab)
            nc.gpsimd.memset(Qh[t][32:33, :].bitcast(F32), 1.0)
            nc.gpsimd.memset(Qh[t][96:97, :].bitcast(F32), 1.0)

        # ---------------- V^T (ones-augmented) ----------------
        # VH[mc][:, 33h:33h+32] = V^T rows mc*128.., head h; col 33h+32 = 1.0
        VH = [ptile(f"VH{mc}", [128, NH * 33], BF16) for mc in range(NH)]
        for mc in range(8):
            pv = psp.tile([128, C], F32, tag="big", bufs=3, name=f"pv_{mc}")
            for kc in range(2):
                nc.tensor.matmul(
                    pv[:],
                    (X[kc][:, mc * 128 : (mc + 1) * 128]),
                    (WVT[kc][:]),
                    start=(kc == 0),
                    stop=(kc == 1),
                )
            nc.vector.memset(VH[mc][:], 1.0)  # ones cols survive the copy below
            vh3 = VH[mc].rearrange("p (h c) -> p h c", c=33)
            nc.vector.tensor_copy(
                vh3[:, :, 0:32], pv.rearrange("p (h d) -> p h d", d=32)
            )

        # ---------------- attention ----------------
        O1u = [ptile(f"O1u{t}", [128, N]) for t in range(2)]
        ESUM = ptile("ESUM", [36, N])  # heads 0-3 rows 0-3; 4-7 rows 32-35
        nc.vector.memset(ESUM[:], 1.0)  # unused rows stay 1.0 -> 1/x finite
        RECIP = ptile("RECIP", [36, N], F32R)
        OH = ptile("OH", [36, C], F32R)
        nc.sync.dma_start(OH[:], io["oh"][:, :])
        for i in range(2):
            sl = slice(i * 128, (i + 1) * 128)
            nc.sync.dma_start(WOT[i][:], io["wot"][sl, :])
            nc.sync.dma_start(XPB[i][:], io["xpb"][sl, :])
        O1 = [ptile(f"O1{t}", [128, N], F32R) for t in range(2)]
        for p in range(4):  # head pairs (2p, 2p+1) in Qh/Kh tile p
            # one [97,512] psum per jn: head A rows 0-32 (array cols 0-32),
            # head B rows 64-96 (array cols 64-96) -> the two AV matmuls run
            # on disjoint column groups concurrently.
            psO = [
                psp.tile([97, 512], F32, tag="psO", bufs=2, name=f"psO_{p}_{jn}")
                for jn in range(2)
            ]
            for mc in range(8):
                for jn in range(2):
                    ps = psp.tile([128, N], F32, tag="big", bufs=3, name=f"ps_{p}_{mc}_{jn}")
                    for hh in range(2):  # array rows 0-32 / 64-96: concurrent
                        base = 64 * hh
                        nc.tensor.matmul(
                            ps[:, hh * 512 : (hh + 1) * 512],
                            (Kh[p][base : base + 33, mc * 128 : (mc + 1) * 128]),
                            (Qh[p][base : base + 33, jn * 512 : (jn + 1) * 512]),
                            start=True,
                            stop=True,
                        )
                    et = etp.tile([128, N], BF16, tag="et", name=f"et_{p}_{mc}_{jn}")
                    nc.scalar.activation(et[:], ps[:], EXP)
                    for hh in range(2):
                        h = 2 * p + hh
                        nc.tensor.matmul(
                            psO[jn][64 * hh : 64 * hh + 33, :],
                            (VH[mc][:, 33 * h : 33 * h + 33]),
                            (et[:, hh * 512 : (hh + 1) * 512]),
                            start=(mc == 0),
                            stop=(mc == 7),
                            tile_position=(0, 64 * hh),
                            skip_group_check=True,
                        )
            for jn in range(2):
                js = slice(jn * 512, (jn + 1) * 512)
                ost = etp.tile([97, 512], F32, tag="ost", bufs=4, name=f"ost_{p}_{jn}")
                nc.vector.tensor_copy(ost[0:33, :], psO[jn][0:33, :])
                nc.vector.tensor_copy(ost[64:97, :], psO[jn][64:97, :])
                for hh in range(2):
                    h = 2 * p + hh
                    t, r = h // 4, 32 * (h % 4)
                    nc.sync.dma_start(
                        O1u[t][r : r + 32, js], ost[64 * hh : 64 * hh + 32, :]
                    )
                    er = 32 * (h // 4) + h % 4
                    nc.sync.dma_start(
                        ESUM[er : er + 1, js], ost[64 * hh + 32 : 64 * hh + 33, :]
                    )



        # ---------------- normalize ----------------
        with nc.allow_low_precision("f32r recip of O(100) sums"):
            nc.vector.reciprocal(RECIP[:], ESUM[:])  # one FD-bound op covers both halves
        for t in range(2):
            rs = slice(32 * t, 32 * t + 4)
            pr = psp.tile([128, N], F32, tag="big", bufs=3, name=f"pr_{t}")
            for jn in range(2):
                js = slice(jn * 512, (jn + 1) * 512)
                nc.tensor.matmul(
                    pr[:, js],
                    (OH[rs, t * 128 : (t + 1) * 128]),
                    (RECIP[rs, js]),
                    start=True,
                    stop=True,
                )
            nc.vector.tensor_mul(O1[t][:], O1u[t][:], pr[:])

        # ---------------- output projection + residual ----------------
        OUTF = [ptile(f"OUTF{t}", [128, N]) for t in range(2)]
        for mo in range(2):
            for jn in range(2):
                js = slice(jn * 512, (jn + 1) * 512)
                po = psp.tile([128, 512], F32, tag="big", bufs=3, name=f"po_{mo}_{jn}")
                for kc in range(2):
                    nc.tensor.matmul(
                        po[:],
                        (WOT[kc][:, mo * 128 : (mo + 1) * 128]),
                        (O1[kc][:, js]),
                        start=(kc == 0),
                        stop=(kc == 1),
                    )
                nc.vector.tensor_add(OUTF[mo][:, js], po[:], XPB[mo][:, js])
            nc.sync.dma_start(io["out"][mo * 128 : (mo + 1) * 128, :], OUTF[mo][:])


def build_nc():
    nc = bacc.Bacc("TRN2", target_bir_lowering=False, debug=False)
    io = {}
    for name, shape, dt_ in [
        ("xb", (C, N), F32R),
        ("xpb", (C, N), F32),
        ("wqt", (C, 512), F32R),
        ("wkt", (C, 512), F32R),
        ("wvt", (C, C), F32R),
        ("wot", (C, C), F32R),
        ("oh", (36, C), F32R),
    ]:
        io[name] = nc.dram_tensor(name, shape, dt_, kind="ExternalInput").ap()
    io["out"] = nc.dram_tensor("out", (C, N), F32, kind="ExternalOutput").ap()
    with tile.TileContext(nc) as tc:
        _emit(tc, io)
    nc.finalize()  # Bacc passes: wait-splitting (1-wait limit), reg alloc
    return nc


def host_prep(x, Wq, bq, Wk, bk, Wv, bv, Wo, bo):
    """Build per-core input maps (numpy only)."""
    x = np.ascontiguousarray(np.asarray(x, np.float32))
    Wq, bq = np.asarray(Wq, np.float32), np.asarray(bq, np.float32)
    Wk = np.asarray(Wk, np.float32)
    Wv, bv = np.asarray(Wv, np.float32), np.asarray(bv, np.float32)
    Wo, bo = np.asarray(Wo, np.float32), np.asarray(bo, np.float32)
    s = 1.0 / math.sqrt(HD)

    wqt = np.zeros((C, 512), np.float32)
    wkt = np.zeros((C, 512), np.float32)
    for h in range(NH):
        hs = slice(HD * h, HD * (h + 1))
        wqt[:, 64 * h : 64 * h + 32] = Wq[hs, :].T * s
        wkt[:, 64 * h : 64 * h + 32] = Wk[hs, :].T
        wkt[:, 64 * h + 32] = (Wk[hs, :].T @ bq[hs]) * s
    wvt = np.ascontiguousarray(Wv.T)
    wot = np.ascontiguousarray(Wo.T)
    bo2 = Wo @ bv + bo
    # oh[32t + j//32, t*128 + j] = 1: selects head rows for the
    # recip-broadcast matmul (RECIP row layout: heads 0-3 at rows 0-3,
    # heads 4-7 at rows 32-35 for partition-aligned matmul slices).
    oh = np.zeros((36, C), np.float32)
    for t in range(2):
        for j in range(128):
            oh[32 * t + j // 32, t * 128 + j] = 1.0

    B = x.shape[0]
    in_maps = []
    for b in range(B):
        xb = np.ascontiguousarray(x[b].reshape(C, N))
        in_maps.append(
            {
                "xb": xb,
                "xpb": np.ascontiguousarray(xb + bo2[:, None]),
                "wqt": wqt,
                "wkt": wkt,
                "wvt": wvt,
                "wot": wot,
                "oh": oh,
            }
        )
    return in_maps


def kernel(x, Wq, bq, Wk, bk, Wv, bv, Wo, bo):
    global _NC, LAST_RESULTS
    if _NC is None:
        _NC = build_nc()
    in_maps = host_prep(x, Wq, bq, Wk, bk, Wv, bv, Wo, bo)
    res = run_bass_kernel_spmd(_NC, in_maps, core_ids=list(range(NCORES)))
    LAST_RESULTS = res
    out = np.stack([r["out"] for r in res.results], axis=0)
    return out.reshape(NCORES, C, 32, 32).astype(np.float32)


if __name__ == "__main__":
    # smoke: random inputs through the kernel
    rng = np.random.default_rng(0)
    ins = {
        "x": rng.standard_normal((8, C, 32, 32), dtype=np.float32),
        "Wq": rng.standard_normal((C, C), dtype=np.float32) / 16,
        "bq": rng.standard_normal(C).astype(np.float32) * 0.01,
        "Wk": rng.standard_normal((C, C), dtype=np.float32) / 16,
        "bk": rng.standard_normal(C).astype(np.float32) * 0.01,
        "Wv": rng.standard_normal((C, C), dtype=np.float32) / 16,
        "bv": rng.standard_normal(C).astype(np.float32) * 0.01,
        "Wo": rng.standard_normal((C, C), dtype=np.float32) / 16,
        "bo": rng.standard_normal(C).astype(np.float32) * 0.01,
    }
    out = kernel(**ins)
    print("out", out.shape, out.dtype, float(np.abs(out).mean()))
